# revision 26
# baseline (speedup 1.0000x reference)
"""LocalAttention (windowed attention, fake-quant) Trainium2 kernel.

Strategy
--------
Wall time is dominated by the ~50 MB/s axon tunnel, so the wire format is
minimized: x ships as int8 (exactly fq(x)'s integer grid, zero accuracy
loss), weights ship as int8 + per-channel scales (exactly fq(W)), the
output returns as fp16.  All matmuls on device run on integer-valued
bf16/fp32 operands so the integer math is bit-exact; the six
data-dependent global fq amaxes are computed on device with tiny
AllReduce(max) collectives (4 barriers).

The Bass program is compiled once and the jitted PJRT callable is cached
in module globals; output device buffers are recycled as donated inputs
so steady-state wire traffic is input(int8+weights) + output(fp16) only.
"""

import numpy as np

DIM = 384
HEADS = 12
WS = 7
HEAD_DIM = DIM // HEADS
SCALE = HEAD_DIM ** -0.5
QMAX = 127.0
QMIN = -128.0
MAGIC = 12582912.0  # 1.5 * 2**23 : float32 round-to-nearest-even trick

B, H, W = 16, 56, 56
R1, R2 = H // WS, W // WS        # 8, 8
N = WS * WS                      # 49
NP2 = 2 * N                      # 98  (two windows packed on partitions)
NCORES = 8
BS = B // NCORES                 # 2 batches per core
SPAT = H * W                     # 3136
T = BS * SPAT                    # 6272 tokens per core
TT = 448                         # token tile for the qkv matmul
NTT = T // TT                    # 14
MW = BS * R1 * R2                # 128 windows per core
NPAIR = MW // 2                  # 64 window pairs per core
C3 = 3 * DIM                     # 1152

_ctx = {}


# --------------------------------------------------------------------------
# host-side exact fake-quant helpers (match reference fq bit-for-bit)
# --------------------------------------------------------------------------

def _fq_int_pertensor(x):
    """Return (int8 grid, scale) with q*scale == fq(x) exactly."""
    am = np.float32(max(np.max(np.abs(x)), 1e-12))
    s = np.float32(am / QMAX)
    buf = x / s          # |buf| <= 127.00002 -> rint stays in [-127, 127]
    np.rint(buf, out=buf)
    return buf.astype(np.int8), s


def _fq_int_perchannel(w):
    """Per-output-channel (axis 0) int grid + scales, w: [O, C]."""
    am = np.maximum(np.abs(w).max(axis=1, keepdims=True), 1e-12).astype(np.float32)
    s = (am / QMAX).astype(np.float32)
    q = np.rint(w / s).astype(np.int8)
    return q, s[:, 0]


def _fq_values(x, axis=None):
    """Plain fq (dequantized values), matching reference."""
    if axis is None:
        am = np.max(np.abs(x))
    else:
        red = tuple(i for i in range(x.ndim) if i != axis)
        am = np.max(np.abs(x), axis=red, keepdims=True)
    am = np.maximum(am, 1e-12).astype(np.float32)
    s = am / QMAX
    return (np.clip(np.round(x / s), QMIN, QMAX) * s).astype(np.float32)


# --------------------------------------------------------------------------
# device program
# --------------------------------------------------------------------------

def _build_bass():
    import concourse.bass as bass
    import concourse.tile as tile
    from concourse import mybir
    from concourse import bass_isa
    from concourse.masks import make_identity
    from contextlib import ExitStack

    f32 = mybir.dt.float32
    bf16 = mybir.dt.bfloat16
    f16 = mybir.dt.float16
    i8 = mybir.dt.int8
    AF = mybir.ActivationFunctionType
    ALU = mybir.AluOpType

    nc = bass.Bass(num_devices=NCORES)

    xq = nc.declare_dram_parameter("xq", [BS, DIM, H, W], i8, isOutput=False)
    wqT = nc.declare_dram_parameter("wqT", [DIM, C3], i8, isOutput=False)
    woT = nc.declare_dram_parameter("woT", [DIM, DIM], i8, isOutput=False)
    avec = nc.declare_dram_parameter("avec", [C3], f32, isOutput=False)
    bvec = nc.declare_dram_parameter("bvec", [C3], f32, isOutput=False)
    swov = nc.declare_dram_parameter("swov", [DIM], f32, isOutput=False)
    bov = nc.declare_dram_parameter("bov", [DIM], f32, isOutput=False)
    biasT = nc.declare_dram_parameter("biasT", [N, HEADS, N], f32, isOutput=False)
    out = nc.declare_dram_parameter("out", [BS, DIM, H, W], f16, isOutput=True)

    av_scr = nc.dram_tensor("av_scr", [NPAIR, NP2, DIM], f32)

    rg = [list(range(NCORES))]

    def AP(t, off, dims):
        return bass.AP(tensor=t.tensor if hasattr(t, "tensor") else t,
                       offset=off, ap=[list(d) for d in dims])

    # window-pair AP pieces. Returns (b, spatial_off, free_dims).
    # SBUF token space offset = b*SPAT + spatial_off (b-stride SPAT);
    # DRAM offset = b*DIM*SPAT + c*SPAT + spatial_off.
    def pair_free(p):
        b = p // (R1 * R2 // 2)
        rem = p % (R1 * R2 // 2)
        r1 = rem // (R2 // 2)
        j = rem % (R2 // 2)
        sp = r1 * WS * H + (2 * j) * WS
        return b, sp, [[WS, 2], [H, WS], [1, WS]]  # (win, h1, w1) -> 98 tokens

    with tile.TileContext(nc) as tc, ExitStack() as ctx:
        const = ctx.enter_context(tc.tile_pool(name="const", bufs=1))
        stats = ctx.enter_context(tc.tile_pool(name="stats", bufs=1))
        dram = ctx.enter_context(tc.tile_pool(name="dram", bufs=1, space="DRAM"))

        # q,k channel scales/biases in 96-channel chunks (3 heads per chunk,
        # so every head starts at partition 0/32/64 — a PE base requirement)
        A96 = const.tile([96, 8], f32)
        nc.sync.dma_start(out=A96, in_=AP(avec[:], 0, [[1, 96], [96, 8]]))
        B96 = const.tile([96, 8], f32)
        nc.sync.dma_start(out=B96, in_=AP(bvec[:], 0, [[1, 96], [96, 8]]))
        swo_sb = const.tile([128, 3], f32)
        nc.sync.dma_start(out=swo_sb, in_=AP(swov[:], 0, [[1, 128], [128, 3]]))
        bo_sb = const.tile([128, 3], f32)
        nc.sync.dma_start(out=bo_sb, in_=AP(bov[:], 0, [[1, 128], [128, 3]]))

        # v-channel scale/bias broadcast tiles [98, 384]
        swv_bc = const.tile([NP2, DIM], f32)
        nc.sync.dma_start(out=swv_bc, in_=AP(avec[:], 2 * DIM, [[0, NP2], [1, DIM]]))
        bv_bc = const.tile([NP2, DIM], f32)
        nc.sync.dma_start(out=bv_bc, in_=AP(bvec[:], 2 * DIM, [[0, NP2], [1, DIM]]))

        # packed (transposed) bias [98, 12, 98]: diag blocks = biasT, cross = 0
        biasP = const.tile([NP2, HEADS, NP2], f32)
        nc.vector.memset(biasP, 0.0)
        nc.sync.dma_start(out=biasP[0:N, :, 0:N], in_=biasT[:, :, :])
        nc.sync.dma_start(out=biasP[N:NP2, :, N:NP2], in_=biasT[:, :, :])

        # 0/1 mask [98, 12, 98] (diag blocks 1)
        mask12 = const.tile([NP2, HEADS, NP2], f32)
        nc.vector.memset(mask12, 0.0)
        nc.vector.memset(mask12[0:N, :, 0:N], 1.0)
        nc.vector.memset(mask12[N:NP2, :, N:NP2], 1.0)

        ident = const.tile([128, 128], bf16)
        make_identity(nc, ident)

        # stat accumulators
        st_qk_hi = stats.tile([96, 8 * NTT], f32)
        st_qk_lo = stats.tile([96, 8 * NTT], f32)
        st_raw = stats.tile([NP2, NPAIR * 3], f32)
        st_v = stats.tile([NP2, NPAIR], f32)
        st_a2 = stats.tile([NP2, NPAIR * 3], f32)
        st_av = stats.tile([NP2, NPAIR], f32)

        # collective bounce buffers
        cc1_in = dram.tile([1, 2], f32)
        cc1_out = dram.tile([1, 2], f32)
        cc2_in = dram.tile([1, 2], f32)
        cc2_out = dram.tile([1, 2], f32)
        cc3_in = dram.tile([1, 1], f32)
        cc3_out = dram.tile([1, 1], f32)
        cc4_in = dram.tile([1, 1], f32)
        cc4_out = dram.tile([1, 1], f32)

        # ---- load weights (int8 staging in a scoped pool) --------------
        wq_bf, wo_bf = [], []
        with tc.tile_pool(name="wstg", bufs=1) as wstg:
            for kc in range(3):
                t_i8 = wstg.tile([128, C3], i8, tag=f"wq8_{kc}", name=f"wq8_{kc}")
                nc.sync.dma_start(out=t_i8, in_=wqT[kc * 128:(kc + 1) * 128, :])
                t_bf = const.tile([128, C3], bf16, tag=f"wqb_{kc}", name=f"wqb_{kc}")
                nc.gpsimd.tensor_copy(out=t_bf, in_=t_i8)
                wq_bf.append(t_bf)
            for kc in range(3):
                t_i8 = wstg.tile([128, DIM], i8, tag=f"wo8_{kc}", name=f"wo8_{kc}")
                nc.sync.dma_start(out=t_i8, in_=woT[kc * 128:(kc + 1) * 128, :])
                t_bf = const.tile([128, DIM], bf16, tag=f"wob_{kc}", name=f"wob_{kc}")
                nc.gpsimd.tensor_copy(out=t_bf, in_=t_i8)
                wo_bf.append(t_bf)

        # ---- q/k integer tiles live until end of P3 --------------------
        qkp_cm = tc.tile_pool(name="qkpool", bufs=1)
        qkp = qkp_cm.__enter__()
        qk_bf = []
        for i in range(8):
            qkt = qkp.tile([96, T], bf16, tag=f"qkr_{i}", name=f"qkr_{i}")
            qk_bf.append(qkt)

        # ---- x (int8 -> bf16, channel-major) lives until end of P2b ----
        xqp_cm = tc.tile_pool(name="xqp", bufs=1)
        xqp = xqp_cm.__enter__()
        xq_bf = []
        with tc.tile_pool(name="xstg", bufs=1) as xstg:
            for kc in range(3):
                t_i8 = xstg.tile([128, BS, SPAT], i8, tag=f"xq8_{kc}",
                                 name=f"xq8_{kc}")
                nc.sync.dma_start(
                    out=t_i8,
                    in_=AP(xq[:], kc * 128 * SPAT,
                           [[SPAT, 128], [DIM * SPAT, BS], [1, SPAT]]))
                t_bf = xqp.tile([128, BS * SPAT], bf16, tag=f"xqb_{kc}",
                                name=f"xqb_{kc}")
                nc.gpsimd.tensor_copy(out=t_bf,
                                      in_=t_i8.rearrange("p b s -> p (b s)"))
                xq_bf.append(t_bf)

        # ======================= P1: qkv pass A (q,k stats) ============
        with tc.tile_pool(name="p1psum", bufs=4, space="PSUM") as pp:
            for oc in range(8):
                for ti in range(NTT):
                    ps = pp.tile([96, TT], f32, tag="qkvps")
                    for kc in range(3):
                        nc.tensor.matmul(
                            ps, lhsT=wq_bf[kc][:, oc * 96:(oc + 1) * 96],
                            rhs=xq_bf[kc][:, ti * TT:(ti + 1) * TT],
                            start=(kc == 0), stop=(kc == 2))
                    col = oc * NTT + ti
                    nc.vector.tensor_reduce(out=st_qk_hi[:, col:col + 1], in_=ps,
                                            op=ALU.max, axis=mybir.AxisListType.X)
                    nc.vector.tensor_reduce(out=st_qk_lo[:, col:col + 1], in_=ps,
                                            op=ALU.min, axis=mybir.AxisListType.X)

        # combine: amax(|A*v+B|) = max(A*hi+B, -(A*lo+B)) ; A>0
        red8h = stats.tile([96, 8], f32)
        red8l = stats.tile([96, 8], f32)
        for oc in range(8):
            nc.vector.tensor_reduce(out=red8h[:, oc:oc + 1],
                                    in_=st_qk_hi[:, oc * NTT:(oc + 1) * NTT],
                                    op=ALU.max, axis=mybir.AxisListType.X)
            nc.vector.tensor_reduce(out=red8l[:, oc:oc + 1],
                                    in_=st_qk_lo[:, oc * NTT:(oc + 1) * NTT],
                                    op=ALU.min, axis=mybir.AxisListType.X)
        nc.vector.tensor_tensor(out=red8h, in0=red8h, in1=A96, op=ALU.mult)
        nc.vector.tensor_tensor(out=red8h, in0=red8h, in1=B96, op=ALU.add)
        nc.vector.tensor_tensor(out=red8l, in0=red8l, in1=A96, op=ALU.mult)
        nc.vector.tensor_tensor(out=red8l, in0=red8l, in1=B96, op=ALU.add)
        nc.vector.tensor_scalar_mul(out=red8l, in0=red8l, scalar1=-1.0)
        nc.vector.tensor_tensor(out=red8h, in0=red8h, in1=red8l, op=ALU.max)
        qk2 = stats.tile([96, 2], f32)
        nc.vector.tensor_reduce(out=qk2[:, 0:1], in_=red8h[:, 0:4],
                                op=ALU.max, axis=mybir.AxisListType.X)
        nc.vector.tensor_reduce(out=qk2[:, 1:2], in_=red8h[:, 4:8],
                                op=ALU.max, axis=mybir.AxisListType.X)
        nc.gpsimd.partition_all_reduce(out_ap=qk2, in_ap=qk2, channels=96,
                                       reduce_op=bass_isa.ReduceOp.max)
        nc.sync.dma_start(out=cc1_in, in_=qk2[0:1, :])
        nc.gpsimd.collective_compute("AllReduce", ALU.max, replica_groups=rg,
                                     ins=[cc1_in[:]], outs=[cc1_out[:]])
        g1 = stats.tile([128, 2], f32)
        nc.sync.dma_start(out=g1, in_=AP(cc1_out, 0, [[0, 128], [1, 2]]))
        nc.vector.tensor_scalar_max(out=g1, in0=g1, scalar1=1e-12)
        g1c = stats.tile([128, 2], f32)   # 127/amax
        nc.vector.reciprocal(out=g1c, in_=g1)
        nc.vector.tensor_scalar_mul(out=g1c, in0=g1c, scalar1=QMAX)
        g1s = stats.tile([128, 2], f32)   # amax/127
        nc.vector.tensor_scalar_mul(out=g1s, in0=g1, scalar1=float(1.0 / QMAX))

        # ======================= P2: qkv pass B -> qr,kr (bf16 ints) ===
        # 8 chunks of 96 channels (3 heads each): tiles 0-3 = q, 4-7 = k
        with tc.tile_pool(name="p2psum", bufs=4, space="PSUM") as pp, \
             tc.tile_pool(name="p2tmp", bufs=4) as tp:
            for oc in range(8):
                cq = g1c[0:96, 0:1] if oc < 4 else g1c[0:96, 1:2]
                for ti in range(NTT):
                    ps = pp.tile([96, TT], f32, tag="qkvps")
                    for kc in range(3):
                        nc.tensor.matmul(
                            ps, lhsT=wq_bf[kc][:, oc * 96:(oc + 1) * 96],
                            rhs=xq_bf[kc][:, ti * TT:(ti + 1) * TT],
                            start=(kc == 0), stop=(kc == 2))
                    t1 = tp.tile([96, TT], f32, tag="t1")
                    nc.scalar.activation(out=t1, in_=ps, func=AF.Identity,
                                         bias=B96[:, oc:oc + 1],
                                         scale=A96[:, oc:oc + 1])
                    t2 = tp.tile([96, TT], f32, tag="t2")
                    nc.vector.tensor_scalar(out=t2, in0=t1, scalar1=cq,
                                            scalar2=MAGIC, op0=ALU.mult, op1=ALU.add)
                    nc.vector.tensor_scalar_add(
                        out=qk_bf[oc][:, ti * TT:(ti + 1) * TT],
                        in0=t2, scalar1=-MAGIC)

        def head_slice(is_k, h, pap):
            # head h lives in chunk h//3 at partition base (h%3)*32
            b, sp, dims = pap
            off = b * SPAT + sp
            t6 = qk_bf[(4 if is_k else 0) + h // 3]
            base = t6[(h % 3) * 32:(h % 3) * 32 + 32, :]
            return AP(base.tensor, base.offset + off, [base.ap[0]] + dims)

        # ======================= P2b: scores pass A + v stats ==========
        with tc.tile_pool(name="sps", bufs=6, space="PSUM") as pp, \
             tc.tile_pool(name="vps", bufs=2, space="PSUM") as vp, \
             tc.tile_pool(name="p2b", bufs=4) as tp:
            for p in range(NPAIR):
                pap = pair_free(p)
                for hg in range(3):
                    ps = pp.tile([NP2, 4 * NP2], f32, tag="sps")
                    for hh in range(4):
                        h = hg * 4 + hh
                        nc.tensor.matmul(
                            ps[:, hh * NP2:(hh + 1) * NP2],
                            lhsT=head_slice(True, h, pap),
                            rhs=head_slice(False, h, pap),
                            start=True, stop=True)
                    msk = tp.tile([NP2, 4 * NP2], f32, tag="msk")
                    nc.vector.tensor_tensor(
                        out=msk, in0=ps,
                        in1=mask12[:, hg * 4:(hg + 1) * 4, :].rearrange("p a b -> p (a b)"),
                        op=ALU.mult)
                    nc.vector.tensor_reduce(out=st_raw[:, p * 3 + hg:p * 3 + hg + 1],
                                            in_=msk, op=ALU.max,
                                            axis=mybir.AxisListType.X,
                                            apply_absolute_value=True)
                # v pass A (stats only)
                ps_v = vp.tile([NP2, DIM], f32, tag="vps")
                for kc in range(3):
                    nc.tensor.matmul(
                        ps_v,
                        lhsT=AP(xq_bf[kc].tensor,
                                xq_bf[kc].offset + pap[0] * SPAT + pap[1],
                                [xq_bf[kc].ap[0]] + pap[2]),
                        rhs=wq_bf[kc][:, 2 * DIM:3 * DIM],
                        start=(kc == 0), stop=(kc == 2))
                tv = tp.tile([NP2, DIM], f32, tag="tv")
                nc.vector.tensor_tensor(out=tv, in0=ps_v, in1=swv_bc, op=ALU.mult)
                nc.vector.tensor_tensor(out=tv, in0=tv, in1=bv_bc, op=ALU.add)
                nc.vector.tensor_reduce(out=st_v[:, p:p + 1], in_=tv, op=ALU.max,
                                        axis=mybir.AxisListType.X,
                                        apply_absolute_value=True)

        xqp_cm.__exit__(None, None, None)   # free xq_bf

        r2 = stats.tile([NP2, 2], f32)
        nc.vector.tensor_reduce(out=r2[:, 0:1], in_=st_raw, op=ALU.max,
                                axis=mybir.AxisListType.X)
        nc.vector.tensor_reduce(out=r2[:, 1:2], in_=st_v, op=ALU.max,
                                axis=mybir.AxisListType.X)
        nc.gpsimd.partition_all_reduce(out_ap=r2, in_ap=r2, channels=NP2,
                                       reduce_op=bass_isa.ReduceOp.max)
        nc.sync.dma_start(out=cc2_in, in_=r2[0:1, :])
        nc.gpsimd.collective_compute("AllReduce", ALU.max, replica_groups=rg,
                                     ins=[cc2_in[:]], outs=[cc2_out[:]])
        g2 = stats.tile([128, 2], f32)
        nc.sync.dma_start(out=g2, in_=AP(cc2_out, 0, [[0, 128], [1, 2]]))
        nc.vector.tensor_scalar_max(out=g2, in0=g2, scalar1=1e-12)
        g2c = stats.tile([128, 2], f32)
        nc.vector.reciprocal(out=g2c, in_=g2)
        nc.vector.tensor_scalar_mul(out=g2c, in0=g2c, scalar1=QMAX)
        # s_a = (g2_raw/127) * s_q * s_k   (scale of fq(attn))
        s_a = stats.tile([128, 1], f32)
        nc.vector.tensor_scalar_mul(out=s_a, in0=g2[:, 0:1],
                                    scalar1=float(1.0 / QMAX))
        nc.vector.tensor_tensor(out=s_a, in0=s_a, in1=g1s[:, 0:1], op=ALU.mult)
        nc.vector.tensor_tensor(out=s_a, in0=s_a, in1=g1s[:, 1:2], op=ALU.mult)
        s_v = stats.tile([128, 1], f32)
        nc.vector.tensor_scalar_mul(out=s_v, in0=g2[:, 1:2],
                                    scalar1=float(1.0 / QMAX))

        # ======================= P3: scores pass B -> r2q (DRAM) + attn2 stats
        r2q_dram = nc.dram_tensor("r2q_scr", [NPAIR, NP2, HEADS * NP2], i8)
        with tc.tile_pool(name="sps2", bufs=6, space="PSUM") as pp, \
             tc.tile_pool(name="p3", bufs=4) as tp, \
             tc.tile_pool(name="p3st", bufs=3) as sp:
            for p in range(NPAIR):
                pap = pair_free(p)
                r2s = sp.tile([NP2, HEADS * NP2], i8, tag="r2s")
                for hg in range(3):
                    ps = pp.tile([NP2, 4 * NP2], f32, tag="sps")
                    for hh in range(4):
                        h = hg * 4 + hh
                        nc.tensor.matmul(
                            ps[:, hh * NP2:(hh + 1) * NP2],
                            lhsT=head_slice(True, h, pap),
                            rhs=head_slice(False, h, pap),
                            start=True, stop=True)
                    msk = tp.tile([NP2, 4 * NP2], f32, tag="msk")
                    nc.vector.tensor_tensor(
                        out=msk, in0=ps,
                        in1=mask12[:, hg * 4:(hg + 1) * 4, :].rearrange("p a b -> p (a b)"),
                        op=ALU.mult)
                    tq = tp.tile([NP2, 4 * NP2], f32, tag="tq")
                    nc.vector.tensor_scalar(out=tq, in0=msk, scalar1=g2c[:NP2, 0:1],
                                            scalar2=MAGIC, op0=ALU.mult, op1=ALU.add)
                    nc.vector.tensor_scalar_add(
                        out=r2s[:, hg * 4 * NP2:(hg + 1) * 4 * NP2],
                        in0=tq, scalar1=-MAGIC)
                    # attn2 = (tq - MAGIC)*s_a + bias ; stats of |attn2|
                    a2 = tp.tile([NP2, 4 * NP2], f32, tag="a2")
                    nc.vector.tensor_scalar(out=a2, in0=tq, scalar1=-MAGIC,
                                            scalar2=s_a[:NP2, 0:1],
                                            op0=ALU.add, op1=ALU.mult)
                    nc.vector.tensor_tensor(
                        out=a2, in0=a2,
                        in1=biasP[:, hg * 4:(hg + 1) * 4, :].rearrange("p a b -> p (a b)"),
                        op=ALU.add)
                    nc.vector.tensor_reduce(out=st_a2[:, p * 3 + hg:p * 3 + hg + 1],
                                            in_=a2, op=ALU.max,
                                            axis=mybir.AxisListType.X,
                                            apply_absolute_value=True)
                nc.sync.dma_start(out=r2q_dram[p, :, :], in_=r2s)

        qkp_cm.__exit__(None, None, None)   # free qk_bf

        r3 = stats.tile([NP2, 1], f32)
        nc.vector.tensor_reduce(out=r3, in_=st_a2, op=ALU.max,
                                axis=mybir.AxisListType.X)
        nc.gpsimd.partition_all_reduce(out_ap=r3, in_ap=r3, channels=NP2,
                                       reduce_op=bass_isa.ReduceOp.max)
        nc.sync.dma_start(out=cc3_in, in_=r3[0:1, :])
        nc.gpsimd.collective_compute("AllReduce", ALU.max, replica_groups=rg,
                                     ins=[cc3_in[:]], outs=[cc3_out[:]])
        g3 = stats.tile([128, 1], f32)
        nc.sync.dma_start(out=g3, in_=AP(cc3_out, 0, [[0, 128], [1, 1]]))
        nc.vector.tensor_scalar_max(out=g3, in0=g3, scalar1=1e-12)
        g3c = stats.tile([128, 1], f32)          # c2 = 127/amax2
        nc.vector.reciprocal(out=g3c, in_=g3)
        nc.vector.tensor_scalar_mul(out=g3c, in0=g3c, scalar1=QMAX)
        g3s = stats.tile([128, 1], f32)          # s2 = amax2/127
        nc.vector.tensor_scalar_mul(out=g3s, in0=g3, scalar1=float(1.0 / QMAX))
        k1 = stats.tile([128, 1], f32)           # s_a * c2
        nc.vector.tensor_tensor(out=k1, in0=s_a, in1=g3c, op=ALU.mult)
        eb = stats.tile([128, 1], f32)           # -(MAGIC*s2 + amax2)
        nc.vector.tensor_scalar_mul(out=eb, in0=g3s, scalar1=-MAGIC)
        nc.vector.tensor_tensor(out=eb, in0=eb, in1=g3, op=ALU.subtract)
        # bias_c2M packed [98,12,98]: diag = bias*c2 + M ; cross = -50*c2 + M
        bc2 = const.tile([NP2, HEADS, NP2], f32, tag="bc2")
        nc.vector.tensor_scalar(out=bc2, in0=biasP, scalar1=g3c[:NP2, 0:1],
                                scalar2=MAGIC, op0=ALU.mult, op1=ALU.add)
        hcM = stats.tile([NP2, 1], f32)
        nc.vector.tensor_scalar(out=hcM, in0=g3c[:NP2, :], scalar1=-50.0,
                                scalar2=MAGIC, op0=ALU.mult, op1=ALU.add)
        nc.vector.tensor_scalar(out=bc2[0:N, :, N:NP2], in0=bc2[0:N, :, N:NP2],
                                scalar1=0.0, scalar2=hcM[0:N, :],
                                op0=ALU.mult, op1=ALU.add)
        nc.vector.tensor_scalar(out=bc2[N:NP2, :, 0:N], in0=bc2[N:NP2, :, 0:N],
                                scalar1=0.0, scalar2=hcM[N:NP2, :],
                                op0=ALU.mult, op1=ALU.add)

        # ======================= P4: e, vr, AV, rinv -> av_scr =========
        with tc.tile_pool(name="avps", bufs=2, space="PSUM") as ap_ps, \
             tc.tile_pool(name="vps2", bufs=2, space="PSUM") as vp, \
             tc.tile_pool(name="p4", bufs=3) as tp, \
             tc.tile_pool(name="p4x", bufs=3) as xp:
            for p in range(NPAIR):
                pap = pair_free(p)
                # -- vr (quantized v, token-major, with ones column per head)
                ps_v = vp.tile([NP2, DIM], f32, tag="vps")
                for kc in range(3):
                    xw_i8 = xp.tile([128, NP2], i8, tag="xw8")
                    for wi in range(2):
                        nc.sync.dma_start(
                            out=xw_i8[:, wi * N:(wi + 1) * N],
                            in_=AP(xq[:],
                                   pap[0] * DIM * SPAT + kc * 128 * SPAT
                                   + pap[1] + wi * WS,
                                   [[SPAT, 128], [H, WS], [1, WS]]))
                    xw_bf = xp.tile([128, NP2], bf16, tag="xwb")
                    nc.gpsimd.tensor_copy(out=xw_bf, in_=xw_i8)
                    nc.tensor.matmul(ps_v, lhsT=xw_bf,
                                     rhs=wq_bf[kc][:, 2 * DIM:3 * DIM],
                                     start=(kc == 0), stop=(kc == 2))
                tv = tp.tile([NP2, DIM], f32, tag="tv")
                nc.vector.tensor_tensor(out=tv, in0=ps_v, in1=swv_bc, op=ALU.mult)
                nc.vector.tensor_tensor(out=tv, in0=tv, in1=bv_bc, op=ALU.add)
                tvq = tp.tile([NP2, DIM], f32, tag="tvq")
                nc.vector.tensor_scalar(out=tvq, in0=tv, scalar1=g2c[:NP2, 1:2],
                                        scalar2=MAGIC, op0=ALU.mult, op1=ALU.add)
                vr = tp.tile([NP2, HEADS, HEAD_DIM + 1], f32, tag="vr")
                nc.vector.tensor_scalar_add(
                    out=AP(vr.tensor, vr.offset,
                           [vr.ap[0], [HEAD_DIM + 1, HEADS], [1, HEAD_DIM]]),
                    in0=tvq, scalar1=-MAGIC)
                nc.vector.memset(
                    AP(vr.tensor, vr.offset + HEAD_DIM,
                       [vr.ap[0], [HEAD_DIM + 1, HEADS], [1, 1]]), 1.0)
                # -- e = exp(round(attn2*c2)*s2 - amax2), cross ~ 0
                r2l = xp.tile([NP2, HEADS * NP2], i8, tag="r2l")
                nc.sync.dma_start(out=r2l, in_=r2q_dram[p, :, :])
                t1 = tp.tile([NP2, HEADS * NP2], f32, tag="t1e")
                nc.vector.tensor_scalar_mul(out=t1, in0=r2l, scalar1=k1[:NP2, :])
                nc.vector.tensor_tensor(
                    out=t1, in0=t1,
                    in1=bc2.rearrange("p a b -> p (a b)"), op=ALU.add)
                e = tp.tile([NP2, HEADS * NP2], f32, tag="e")
                nc.scalar.activation(out=e, in_=t1, func=AF.Exp,
                                     bias=eb[:NP2, :], scale=g3s[:NP2, :])
                # -- AV + column sums
                ps_av = ap_ps.tile([NP2, HEADS * (HEAD_DIM + 1)], f32, tag="avps")
                for h in range(HEADS):
                    nc.tensor.matmul(
                        ps_av[:, h * (HEAD_DIM + 1):(h + 1) * (HEAD_DIM + 1)],
                        lhsT=e[:, h * NP2:(h + 1) * NP2],
                        rhs=vr[:, h, :],
                        start=True, stop=True)
                sums = tp.tile([NP2, HEADS], f32, tag="sums")
                nc.vector.reciprocal(
                    out=sums,
                    in_=AP(ps_av.tensor, ps_av.offset + HEAD_DIM,
                           [ps_av.ap[0], [HEAD_DIM + 1, HEADS], [1, 1]]))
                nc.vector.tensor_scalar_mul(out=sums, in0=sums, scalar1=s_v[:NP2, :])
                rb = tp.tile([NP2, HEADS, HEAD_DIM], f32, tag="rb")
                nc.sync.dma_start(
                    out=rb,
                    in_=AP(sums.tensor, sums.offset,
                           [sums.ap[0], [1, HEADS], [0, HEAD_DIM]]))
                avs = tp.tile([NP2, HEADS * HEAD_DIM], f32, tag="avs")
                nc.vector.tensor_tensor(
                    out=avs,
                    in0=AP(ps_av.tensor, ps_av.offset,
                           [ps_av.ap[0], [HEAD_DIM + 1, HEADS], [1, HEAD_DIM]]),
                    in1=rb.rearrange("p a b -> p (a b)"), op=ALU.mult)
                nc.vector.tensor_reduce(out=st_av[:, p:p + 1], in_=avs, op=ALU.max,
                                        axis=mybir.AxisListType.X,
                                        apply_absolute_value=True)
                nc.sync.dma_start(out=av_scr[p, :, :], in_=avs)

        r4s = stats.tile([NP2, 1], f32)
        nc.vector.tensor_reduce(out=r4s, in_=st_av, op=ALU.max,
                                axis=mybir.AxisListType.X)
        nc.gpsimd.partition_all_reduce(out_ap=r4s, in_ap=r4s, channels=NP2,
                                       reduce_op=bass_isa.ReduceOp.max)
        nc.sync.dma_start(out=cc4_in, in_=r4s[0:1, :])
        nc.gpsimd.collective_compute("AllReduce", ALU.max, replica_groups=rg,
                                     ins=[cc4_in[:]], outs=[cc4_out[:]])
        g4 = stats.tile([128, 1], f32)
        nc.sync.dma_start(out=g4, in_=AP(cc4_out, 0, [[0, 128], [1, 1]]))
        nc.vector.tensor_scalar_max(out=g4, in0=g4, scalar1=1e-12)
        g4c = stats.tile([128, 1], f32)
        nc.vector.reciprocal(out=g4c, in_=g4)
        nc.vector.tensor_scalar_mul(out=g4c, in0=g4c, scalar1=QMAX)
        # per-out-channel projection scale = swo * s3
        pscale = stats.tile([128, 3], f32)
        g4s = stats.tile([128, 1], f32)
        nc.vector.tensor_scalar_mul(out=g4s, in0=g4, scalar1=float(1.0 / QMAX))
        nc.vector.tensor_scalar_mul(out=pscale, in0=swo_sb, scalar1=g4s)

        # ======================= P5: fq(av) -> transpose -> proj -> out
        with tc.tile_pool(name="tps", bufs=3, space="PSUM") as tpp, \
             tc.tile_pool(name="ops", bufs=3, space="PSUM") as opp, \
             tc.tile_pool(name="p5", bufs=3) as tp:
            for p in range(NPAIR):
                pap = pair_free(p)
                av_in = tp.tile([NP2, DIM], f32, tag="avin")
                nc.sync.dma_start(out=av_in, in_=av_scr[p, :, :])
                tq = tp.tile([NP2, DIM], f32, tag="tq5")
                nc.vector.tensor_scalar(out=tq, in0=av_in, scalar1=g4c[:NP2, :],
                                        scalar2=MAGIC, op0=ALU.mult, op1=ALU.add)
                r4 = tp.tile([NP2, DIM], bf16, tag="r45")
                nc.vector.tensor_scalar_add(out=r4, in0=tq, scalar1=-MAGIC)
                avT = []
                for cc in range(3):
                    pst = tpp.tile([128, NP2], bf16, tag="tpps")
                    nc.tensor.transpose(pst, in_=r4[:, cc * 128:(cc + 1) * 128],
                                        identity=ident[0:NP2, 0:NP2])
                    sb = tp.tile([128, NP2], bf16, tag=f"avT{cc}")
                    nc.scalar.copy(out=sb, in_=pst)
                    avT.append(sb)
                for oc in range(3):
                    pso = opp.tile([128, NP2], f32, tag="ops")
                    for cc in range(3):
                        nc.tensor.matmul(pso,
                                         lhsT=wo_bf[cc][:, oc * 128:(oc + 1) * 128],
                                         rhs=avT[cc], start=(cc == 0), stop=(cc == 2))
                    of = tp.tile([128, NP2], f16, tag="of")
                    nc.vector.tensor_scalar(out=of, in0=pso,
                                            scalar1=pscale[:, oc:oc + 1],
                                            scalar2=bo_sb[:, oc:oc + 1],
                                            op0=ALU.mult, op1=ALU.add)
                    for wi in range(2):
                        nc.sync.dma_start(
                            out=AP(out[:],
                                   pap[0] * DIM * SPAT + oc * 128 * SPAT
                                   + pap[1] + wi * WS,
                                   [[SPAT, 128], [H, WS], [1, WS]]),
                            in_=of[:, wi * N:(wi + 1) * N])

    return nc


# --------------------------------------------------------------------------
# cached PJRT runner (replicates run_bass_via_pjrt's multi-core path, but
# keeps the jitted callable + donated output buffers across calls)
# --------------------------------------------------------------------------

def _get_runner():
    if "run" in _ctx:
        return _ctx["run"]

    import jax
    from jax.sharding import Mesh, PartitionSpec
    from jax.experimental.shard_map import shard_map
    from concourse import mybir
    from concourse.bass2jax import (_bass_exec_p, partition_id_tensor,
                                    install_neuronx_cc_hook)

    nc = _build_bass()
    install_neuronx_cc_hook()

    partition_name = (nc.partition_id_tensor.name
                      if nc.partition_id_tensor else None)
    in_names, out_names, out_avals, zero_outs = [], [], [], []
    for alloc in nc.m.functions[0].allocations:
        if not isinstance(alloc, mybir.MemoryLocationSet):
            continue
        name = alloc.memorylocations[0].name
        if alloc.kind == "ExternalInput":
            if name != partition_name:
                in_names.append(name)
        elif alloc.kind == "ExternalOutput":
            shape = tuple(alloc.tensor_shape)
            dtype = mybir.dt.np(alloc.dtype)
            out_names.append(name)
            out_avals.append(jax.core.ShapedArray(shape, dtype))
            zero_outs.append(np.zeros((NCORES * shape[0],) + shape[1:], dtype))
    n_params = len(in_names)
    n_outs = len(out_avals)
    all_names = list(in_names) + list(out_names)
    if partition_name is not None:
        all_names.append(partition_name)
    donate = tuple(range(n_params, n_params + n_outs))

    def _body(*args):
        operands = list(args)
        if partition_name is not None:
            operands.append(partition_id_tensor())
        outs = _bass_exec_p.bind(
            *operands,
            out_avals=tuple(out_avals),
            in_names=tuple(all_names),
            out_names=tuple(out_names),
            lowering_input_output_aliases=(),
            sim_require_finite=True,
            sim_require_nnan=True,
            nc=nc,
        )
        return tuple(outs)

    devices = jax.devices()[:NCORES]
    mesh = Mesh(np.asarray(devices), ("core",))
    in_specs = (PartitionSpec("core"),) * (n_params + n_outs)
    out_specs = (PartitionSpec("core"),) * n_outs
    sharded = jax.jit(
        shard_map(_body, mesh=mesh, in_specs=in_specs, out_specs=out_specs,
                  check_rep=False),
        donate_argnums=donate, keep_unused=True)

    state = {"donate": zero_outs}

    def run(concat_in_by_name):
        args = [concat_in_by_name[n] for n in in_names]
        outs = sharded(*args, *state["donate"])
        host = [np.asarray(o) for o in outs]
        state["donate"] = list(outs)
        return dict(zip(out_names, host))

    _ctx["run"] = run
    return run


# --------------------------------------------------------------------------
# host wrapper
# --------------------------------------------------------------------------

def _prep_static(Wqkv, bqkv, Wout, bout, bias_table, rel_idx):
    wq8, sw = _fq_int_perchannel(Wqkv)          # [1152,384] int8, [1152]
    wo8, swo = _fq_int_perchannel(Wout)         # [384,384] int8, [384]
    coords = np.stack(np.meshgrid(np.arange(WS), np.arange(WS), indexing="ij"))
    bias = bias_table[np.asarray(rel_idx).reshape(-1)]
    bias = bias.reshape(N, N, HEADS).transpose(2, 0, 1).astype(np.float32)
    bias_fq = _fq_values(bias, axis=0)          # [12,49,49]
    biasT = np.ascontiguousarray(bias_fq.transpose(2, 0, 1))  # [n,h,m]=[49,12,49]
    return wq8, sw, wo8, swo, biasT


def _device_forward(x, Wqkv, bqkv, Wout, bout, bias_table, rel_idx):
    run = _get_runner()

    am_x = np.float32(max(np.max(np.abs(x)), 1e-12))
    sx = np.float32(am_x / QMAX)
    buf = x / sx
    np.rint(buf, out=buf)
    xq8 = buf.astype(np.int8)                   # [16,384,56,56]

    wq8, sw, wo8, swo, biasT = _prep_static(Wqkv, bqkv, Wout, bout,
                                            bias_table, rel_idx)
    scale_rows = np.where(np.arange(C3) < DIM, np.float32(SCALE), np.float32(1.0))
    A = (sx * sw * scale_rows).astype(np.float32)
    Bv = (bqkv * scale_rows).astype(np.float32)

    wqT8 = np.ascontiguousarray(wq8.T)          # [384,1152]
    woT8 = np.ascontiguousarray(wo8.T)          # [384,384]

    ins = {
        "xq": xq8,                                      # concat axis0 == batches
        "wqT": np.tile(wqT8, (NCORES, 1)),
        "woT": np.tile(woT8, (NCORES, 1)),
        "avec": np.tile(A, NCORES),
        "bvec": np.tile(Bv, NCORES),
        "swov": np.tile(swo.astype(np.float32), NCORES),
        "bov": np.tile(bout.astype(np.float32), NCORES),
        "biasT": np.tile(biasT, (NCORES, 1, 1)),
    }
    outs = run(ins)
    o = outs["out"]                              # [16,384,56,56] fp16
    return o.astype(np.float32)


# --------------------------------------------------------------------------
# numpy fallback (exact but slow) — only used if the device path fails
# --------------------------------------------------------------------------

def _host_reference(x, Wqkv, bqkv, Wout, bout, bias_table, rel_idx):
    r1, r2 = H // WS, W // WS
    xw = (x.reshape(B, DIM, r1, WS, r2, WS).transpose(0, 2, 4, 3, 5, 1)
          .reshape(B * r1 * r2, N, DIM))
    bias = bias_table[np.asarray(rel_idx).reshape(-1)]
    bias = bias.reshape(N, N, HEADS).transpose(2, 0, 1)
    bias_q = _fq_values(bias.astype(np.float32), axis=0)
    fxw = _fq_values(xw)
    fWq = _fq_values(Wqkv, axis=0)
    qkv = np.matmul(fxw.reshape(-1, DIM), fWq.T) + bqkv
    qkv = qkv.reshape(B * r1 * r2, N, 3 * DIM).astype(np.float32)
    q, k, v = qkv[:, :, :DIM], qkv[:, :, DIM:2 * DIM], qkv[:, :, 2 * DIM:]
    q = _fq_values(q * SCALE)
    k = _fq_values(k)
    v = _fq_values(v)

    def to_heads(t):
        return t.reshape(t.shape[0], N, HEADS, HEAD_DIM).transpose(0, 2, 1, 3)

    q, k, v = to_heads(q), to_heads(k), to_heads(v)
    attn = np.matmul(q, k.transpose(0, 1, 3, 2))
    attn = _fq_values(attn) + bias_q[None]
    attn = _fq_values(attn)
    m = np.max(attn, axis=3, keepdims=True)
    e = np.exp(attn - m)
    p = e / np.sum(e, axis=3, keepdims=True)
    o = np.matmul(p, v)
    o = o.transpose(0, 2, 1, 3).reshape(B * r1 * r2, N, DIM)
    o = np.matmul(_fq_values(o).reshape(-1, DIM), _fq_values(Wout, axis=0).T) + bout
    o = o.reshape(B, r1, r2, WS, WS, DIM).transpose(0, 5, 1, 3, 2, 4)
    return np.ascontiguousarray(o.reshape(B, DIM, H, W)).astype(np.float32)


def kernel(x, Wqkv, bqkv, Wout, bout, bias_table, rel_idx):
    x = np.asarray(x, np.float32)
    Wqkv = np.asarray(Wqkv, np.float32)
    bqkv = np.asarray(bqkv, np.float32)
    Wout = np.asarray(Wout, np.float32)
    bout = np.asarray(bout, np.float32)
    bias_table = np.asarray(bias_table, np.float32)
    try:
        return _device_forward(x, Wqkv, bqkv, Wout, bout, bias_table, rel_idx)
    except Exception:
        import traceback
        traceback.print_exc()
        return _host_reference(x, Wqkv, bqkv, Wout, bout, bias_table, rel_idx)
